# revision 4
# baseline (speedup 1.0000x reference)
"""Trainium2 Bass kernel: causal self-attention (B=4, T=2048, D=1024, H=16).

Sharding: 8 cores = (batch b in 0..3) x (head-group hg in 0..1).
Each core handles one batch element and 8 heads (CL=512 channels).

v2 design (vs v1):
  - all matmul operands bf16 (host converts x/W); PSUM accumulation stays fp32
  - x transposed by the DMA xbar (bf16 path, 128 partitions) straight into
    SBUF — no PE transposes, no PSUM round-trip
  - one continuous software pipeline: the projections for query-chunk m+1
    (xT dma, V, q/k) and the output projection of chunk m-1 are drained as
    PE "filler" inside attention chunk m's jj loop, where the PE would
    otherwise stall on the scalar-engine exp
  - softmax denominators: both heads of a pair batched into one [2,512]
    Ln + one Exp on ACT; numerators evacuated to AT immediately (ACT copy)
    so the PSUM otp banks free fast, normalization happens in-place later
Attention math per head-pair ct (heads 2ct, 2ct+1), query chunk m:
  ST = kT x qT packed in disjoint PE row halves -> exp -> tri mask (diag)
  -> PV accumulate with a ones column giving the denominator row for free.
Host combines: y[b] = (yTp[2b] + yTp[2b+1])^T + b_out.
"""

import numpy as np

B, T, D = 4, 2048, 1024
H, DH = 16, 64
HL, CL = 8, 512          # local heads / channels per core
NT = T // 128            # 16 token tiles
NKT = D // 128           # 8 contraction tiles for QKV
NM = T // 512            # 4 query chunks

_CACHE = {}


def build_program(reps=1, timing=False):
    import concourse.bacc as bacc
    import concourse.tile as tile
    from concourse import mybir

    F32 = mybir.dt.float32
    F16 = mybir.dt.float16
    AF = mybir.ActivationFunctionType

    nc = bacc.Bacc("TRN2", target_bir_lowering=False, debug=False)

    # weights arrive host-pre-arranged to match the SBUF tile layouts so the
    # loads are single contiguous descriptor chains
    xb = nc.dram_tensor("xb", [T, D], F16, kind="ExternalInput")
    wq = nc.dram_tensor("wq", [128, NKT, CL], F16, kind="ExternalInput")
    wk = nc.dram_tensor("wk", [128, NKT, CL], F16, kind="ExternalInput")
    wv = nc.dram_tensor("wv", [128, NKT, CL], F16, kind="ExternalInput")
    wo = nc.dram_tensor("wo", [128, 4, D], F16, kind="ExternalInput")
    bqk = nc.dram_tensor("bqk", [128, 8], F32, kind="ExternalInput")
    bva = nc.dram_tensor("bva", [HL * 65], F32, kind="ExternalInput")
    tri = nc.dram_tensor("tri", [128, 128], F16, kind="ExternalInput")
    # output in block-major layout [cot, m, 128, 512] so each store is one
    # contiguous descriptor chain; host reassembles to [D, T]
    if timing:
        ytp = nc.dram_tensor("ytp", [8, NM, 128, 512], F32)
        done = nc.dram_tensor("done", [1, 4], F32, kind="ExternalOutput")
    else:
        ytp = nc.dram_tensor("ytp", [8, NM, 128, 512], F32,
                             kind="ExternalOutput")
        done = None

    with tile.TileContext(nc) as tc:
        with tc.tile_pool(name="consts", bufs=1) as consts, \
             tc.tile_pool(name="wts", bufs=1) as wts, \
             tc.tile_pool(name="xt", bufs=1) as xtpool, \
             tc.tile_pool(name="qk", bufs=1) as qkpool, \
             tc.tile_pool(name="va", bufs=1) as vapool, \
             tc.tile_pool(name="at", bufs=1) as atpool, \
             tc.tile_pool(name="pt", bufs=4) as ptpool, \
             tc.tile_pool(name="rows", bufs=8) as rows, \
             tc.tile_pool(name="drows", bufs=4) as drows, \
             tc.tile_pool(name="rr", bufs=4) as rrpool, \
             tc.tile_pool(name="bcsp", bufs=4) as bcspool, \
             tc.tile_pool(name="oy", bufs=4) as oypool, \
             tc.tile_pool(name="psS", bufs=2, space="PSUM") as psS, \
             tc.tile_pool(name="psO", bufs=2, space="PSUM") as psO, \
             tc.tile_pool(name="psF", bufs=2, space="PSUM") as psF:

            # Pin the ACT table set holding Copy+Ln+Exp (avoids ~2.7us
            # per-call set switches).
            nc.scalar.add_instruction(mybir.InstLoadActFuncSet(
                act_func_set_id=6,
                name=nc.get_next_instruction_name(),
                ins=[], outs=[]))
            tri_sb = consts.tile([128, 128], F16)
            nc.sync.dma_start(out=tri_sb, in_=tri[:])
            bqk_sb = consts.tile([128, 8], F32)
            nc.scalar.dma_start(out=bqk_sb, in_=bqk[:])
            bq_sb, bk_sb = bqk_sb[:, 0:4], bqk_sb[:, 4:8]
            bva_row = consts.tile([1, HL * 65], F32)
            nc.scalar.dma_start(out=bva_row, in_=bva[:].unsqueeze(0))
            bvat = consts.tile([128, HL * 65], F32)
            nc.gpsimd.partition_broadcast(bvat, bva_row)

            # denominator gather tiles: 4 rows per tile at the partition
            # bases partition_broadcast accepts (0/32/64/96); 2 tiles per
            # chunk, double-buffered by chunk parity; memset once so the
            # untouched partitions stay finite under Ln
            den4 = [[consts.tile([97, 512], F16, name=f"dn{p}{i}",
                                 tag=f"dn{p}{i}") for i in range(2)]
                    for p in range(2)]
            ln4 = [[consts.tile([97, 512], F32, name=f"ln{p}{i}",
                                tag=f"ln{p}{i}") for i in range(2)]
                   for p in range(2)]
            rc4 = [[consts.tile([97, 512], F16, name=f"rc{p}{i}",
                                tag=f"rc{p}{i}") for i in range(2)]
                   for p in range(2)]
            for p in range(2):
                for i in range(2):
                    nc.vector.memset(den4[p][i], 1.0)

            # persistent activations
            xT = [xtpool.tile([128, 4, T], F16, name=f"xT{g}", tag=f"xT{g}")
                  for g in range(2)]
            qT = [qkpool.tile([128, T], F16, name=f"qT{c}", tag=f"qT{c}")
                  for c in range(4)]
            kT = [qkpool.tile([128, T], F16, name=f"kT{c}", tag=f"kT{c}")
                  for c in range(4)]
            vA = [vapool.tile([128, HL * 65], F16, name=f"vA{t}", tag=f"vA{t}")
                  for t in range(NT)]
            AT = [atpool.tile([128, T], F16, name=f"AT{c}", tag=f"AT{c}")
                  for c in range(4)]
            # weights (per rep reload, matching the graded single-shot call)
            wq_sb = wts.tile([128, NKT, CL], F16, tag="wq")
            wk_sb = wts.tile([128, NKT, CL], F16, tag="wk")
            wv_sb = wts.tile([128, NKT, CL], F16, tag="wv")
            wo_sb = wts.tile([128, 4, D], F16, tag="wo")

            for _rep in range(reps):
                # ---------------- DMA emissions ----------------
                def emit_xdma(mm, split=False):
                    # transpose x token-chunk mm into xT via the DMA xbar
                    for g2 in range(NKT):
                        eng = nc.scalar if (split and g2 % 2) else nc.sync
                        eng.dma_start(
                            out=xT[g2 // 4][:, g2 % 4, mm * 512:(mm + 1) * 512],
                            in_=xb[mm * 512:(mm + 1) * 512,
                                   g2 * 128:(g2 + 1) * 128],
                            transpose=True)

                # ---------------- PE work units ----------------
                def emit_V(tt):
                    psv = psF.tile([128, 512], F32, tag="psF")
                    for kt in range(NKT):
                        nc.tensor.matmul(
                            psv,
                            xT[kt // 4][:, kt % 4, tt * 128:(tt + 1) * 128],
                            wv_sb[:, kt, :],
                            start=(kt == 0), stop=(kt == NKT - 1))
                    nc.vector.tensor_tensor(
                        out=vA[tt].rearrange("p (a b) -> p a b",
                                             b=65)[:, :, 0:64],
                        in0=psv.rearrange("p (a b) -> p a b", a=HL),
                        in1=bvat.rearrange("p (a b) -> p a b",
                                           b=65)[:, :, 0:64],
                        op=mybir.AluOpType.add)

                def emit_qk(ct, mm, isq):
                    ps = psF.tile([128, 512], F32, tag="psF")
                    wsrc = wq_sb if isq else wk_sb
                    for kt in range(NKT):
                        nc.tensor.matmul(
                            ps,
                            wsrc[:, kt, ct * 128:(ct + 1) * 128],
                            xT[kt // 4][:, kt % 4, mm * 512:(mm + 1) * 512],
                            start=(kt == 0), stop=(kt == NKT - 1))
                    dst = qT[ct] if isq else kT[ct]
                    bias = (bq_sb if isq else bk_sb)[:, ct:ct + 1]
                    nc.vector.tensor_scalar_add(
                        out=dst[:, mm * 512:(mm + 1) * 512],
                        in0=ps, scalar1=bias)

                def emit_D(mm, cot):
                    psy = psF.tile([128, 512], F32, tag="psF")
                    for cc in range(4):
                        nc.tensor.matmul(
                            psy,
                            wo_sb[:, cc, cot * 128:(cot + 1) * 128],
                            AT[cc][:, mm * 512:(mm + 1) * 512],
                            start=(cc == 0), stop=(cc == 3))
                    oy = oypool.tile([128, 512], F32, tag="oy")
                    nc.vector.tensor_copy(oy, psy)
                    nc.sync.dma_start(out=ytp[cot, mm], in_=oy)

                filler = []
                fdone = set()

                def drain(n):
                    for _ in range(min(n, len(filler))):
                        key, fn = filler.pop(0)
                        fdone.add(key)
                        fn()

                def drain_until(key):
                    # run filler units front-first until `key` has executed;
                    # emission-order safety: producers must drain before
                    # their consumers are emitted on the same engine queue
                    if key in fdone:
                        return
                    assert any(k == key for k, _ in filler), key
                    while True:
                        k, fn = filler.pop(0)
                        fdone.add(k)
                        fn()
                        if k == key:
                            return

                # ---------------- prologue ----------------
                # sync: wq then x-chunk evens; scalar: x-chunk odds then the
                # weights needed progressively later
                nc.sync.dma_start(out=wq_sb, in_=wq[:])
                emit_xdma(0, split=True)
                nc.scalar.dma_start(out=wv_sb, in_=wv[:])
                nc.sync.dma_start(out=wk_sb, in_=wk[:])
                emit_xdma(1, split=True)
                nc.scalar.dma_start(out=wo_sb, in_=wo[:])
                for tt in range(NT):
                    nc.gpsimd.memset(
                        vA[tt].rearrange("p (a b) -> p a b",
                                         b=65)[:, :, 64:65], 1.0)
                # only qk0(ct=0) runs up-front; V0..3 and the other qk0
                # groups become filler under attention (m=0, ct=0)
                emit_qk(0, 0, True)
                emit_qk(0, 0, False)
                fdone.add(("qk", 0, 0))
                for tt in range(4):
                    filler.append((("V", tt), lambda tt=tt: emit_V(tt)))
                for ct in range(1, 4):
                    filler.append(
                        (("qk", 0, ct), lambda ct=ct: (emit_qk(ct, 0, True),
                                                       emit_qk(ct, 0, False))))

                # ---------------- main pipeline over query chunks ----------
                for m in range(NM):
                    if m + 2 < NM:
                        emit_xdma(m + 2)
                    if m + 1 < NM:
                        for tt in range(4 * (m + 1), 4 * (m + 1) + 4):
                            filler.append(
                                (("V", tt), lambda tt=tt: emit_V(tt)))
                        for ct in range(4):
                            filler.append(
                                (("qk", m + 1, ct),
                                 lambda ct=ct, mm=m + 1: (
                                     emit_qk(ct, mm, True),
                                     emit_qk(ct, mm, False))))
                    # output-projection fillers, rebalanced toward the
                    # filler-starved last chunk: D0@m1, D1 split m2/m3, D2@m3
                    dunits = {1: [(0, c) for c in range(8)],
                              2: [(1, c) for c in range(4)],
                              3: [(1, c) for c in range(4, 8)] +
                                 [(2, c) for c in range(8)]}.get(m, [])
                    for mm, cot in dunits:
                        filler.append(
                            (("D", mm, cot),
                             lambda mm=mm, cot=cot: emit_D(mm, cot)))

                    mq = m * 512
                    njj = 4 * m + 4
                    dgt = den4[m % 2]
                    scrs = [None] * 8

                    def jpos(jj):
                        if jj < 4:
                            return 4 * m + jj, jj * 128
                        return jj - 4, 0

                    def emit_ST(jj):
                        j, qoff = jpos(jj)
                        js = slice(j * 128, (j + 1) * 128)
                        stD = psS.tile([128, 1024], F32, tag="psS")
                        nc.tensor.matmul(
                            stD[:, qoff:512],
                            kT[ct][0:64, js],
                            qT[ct][0:64, mq + qoff:mq + 512],
                            start=True, stop=True)
                        nc.tensor.matmul(
                            stD[:, 512 + qoff:1024],
                            kT[ct][64:128, js],
                            qT[ct][64:128, mq + qoff:mq + 512],
                            start=True, stop=True)
                        return stD

                    for ct in range(4):
                        drain_until(("qk", m, ct))
                        otpA = psO.tile([65, 512], F32, tag="psO")
                        otpB = psO.tile([65, 512], F32, tag="psO")
                        pend = emit_ST(0)
                        for jj in range(njj):
                            diag = jj < 4
                            j, qoff = jpos(jj)
                            stD = pend
                            ptD = ptpool.tile([128, 1024], F16, tag="pt")
                            if diag:
                                pt3 = ptD.rearrange("p (h q) -> p h q", h=2)
                                st3 = stD.rearrange("p (h q) -> p h q", h=2)
                                nc.scalar.activation(
                                    out=pt3[:, :, qoff:512],
                                    in_=st3[:, :, qoff:512],
                                    func=AF.Exp, scale=0.125)
                                nc.vector.tensor_mul(
                                    pt3[:, :, qoff:qoff + 128],
                                    pt3[:, :, qoff:qoff + 128],
                                    tri_sb.unsqueeze(1).broadcast_to(
                                        [128, 2, 128]))
                            else:
                                nc.scalar.activation(
                                    out=ptD, in_=stD,
                                    func=AF.Exp, scale=0.125)
                            if diag:
                                drain_until(("V", j))
                            if jj + 1 < njj:
                                pend = emit_ST(jj + 1)
                            drain(1)
                            hA, hB = 2 * ct, 2 * ct + 1
                            nc.tensor.matmul(
                                otpA[:, qoff:512],
                                vA[j][:, hA * 65:(hA + 1) * 65],
                                ptD[:, qoff:512],
                                start=(jj == 0), stop=(jj == njj - 1))
                            nc.tensor.matmul(
                                otpB[:, qoff:512],
                                vA[j][:, hB * 65:(hB + 1) * 65],
                                ptD[:, 512 + qoff:1024],
                                start=(jj == 0), stop=(jj == njj - 1))
                        # evacuate numerator+denominator in one copy per head
                        # (frees the psO banks fast); denominator rows gather
                        # into den8 so one Ln+Exp serves all 8 heads of m
                        scrA = rows.tile([65, 512], F16, tag="scr")
                        scrB = rows.tile([65, 512], F16, tag="scr")
                        nc.vector.tensor_copy(scrA, otpA)
                        nc.vector.tensor_copy(scrB, otpB)
                        scrs[2 * ct], scrs[2 * ct + 1] = scrA, scrB
                        for i, scr in ((2 * ct, scrA), (2 * ct + 1, scrB)):
                            nc.sync.dma_start(
                                out=dgt[i // 4][32 * (i % 4):32 * (i % 4) + 1,
                                                :],
                                in_=scr[64:65, :])
                        drain(2)
                    for i in range(2):
                        nc.scalar.activation(out=ln4[m % 2][i], in_=dgt[i],
                                             func=AF.Ln)
                        nc.scalar.activation(out=rc4[m % 2][i],
                                             in_=ln4[m % 2][i],
                                             func=AF.Exp, scale=-1.0)
                    for h in range(8):
                        # partition_broadcast only reads partition 0 on HW;
                        # hop nonzero-base rows into a base-0 tile first
                        src = rc4[m % 2][h // 4]
                        if h % 4 == 0:
                            row = src[0:1, :]
                        else:
                            rr = rrpool.tile([1, 512], F16, tag="rr")
                            nc.sync.dma_start(
                                out=rr, in_=src[32 * (h % 4):
                                                32 * (h % 4) + 1, :])
                            row = rr
                        bcs = bcspool.tile([64, 512], F16, tag="bcs")
                        nc.gpsimd.partition_broadcast(bcs, row)
                        nc.vector.tensor_mul(
                            AT[h // 2][64 * (h % 2):64 * (h % 2) + 64,
                                       mq:mq + 512],
                            scrs[h][0:64, :], bcs)
                    drain(len(filler))

                for cot in range(8):
                    emit_D(NM - 1, cot)
            if done is not None:
                dn = consts.tile([1, 4], F32)
                nc.vector.memset(dn, 1.0)
                nc.sync.dma_start(out=done[:], in_=dn)
    nc.compile()
    return nc


def make_in_maps(x, W_qkv, b_qkv, W_out):
    x = np.asarray(x, dtype=np.float32)
    W_qkv = np.asarray(W_qkv, dtype=np.float32)
    b_qkv = np.asarray(b_qkv, dtype=np.float32)
    W_out = np.asarray(W_out, dtype=np.float32)
    tri = np.asarray(
        np.arange(128)[None, :] >= np.arange(128)[:, None], dtype=np.float16)

    def warr(w):  # [D, CL] -> [128, NKT, CL] (partition-major tile layout)
        return np.ascontiguousarray(
            w.reshape(NKT, 128, CL).transpose(1, 0, 2)).astype(np.float16)

    in_maps = []
    for core in range(8):
        b, hg = core // 2, core % 2
        cs = hg * CL
        bv = b_qkv[2 * D + cs:2 * D + cs + CL]
        bva = np.zeros(HL * 65, dtype=np.float32)
        bva.reshape(HL, 65)[:, 0:64] = bv.reshape(HL, 64)
        in_maps.append({
            "xb": np.ascontiguousarray(x[b]).astype(np.float16),
            "wq": warr(W_qkv[:, cs:cs + CL]),
            "wk": warr(W_qkv[:, D + cs:D + cs + CL]),
            "wv": warr(W_qkv[:, 2 * D + cs:2 * D + cs + CL]),
            "wo": np.ascontiguousarray(
                W_out[cs:cs + CL, :].reshape(4, 128, D)
                .transpose(1, 0, 2)).astype(np.float16),
            "bqk": np.ascontiguousarray(np.concatenate([
                b_qkv[cs:cs + CL].reshape(4, 128).T,
                b_qkv[D + cs:D + cs + CL].reshape(4, 128).T], axis=1)),
            "bva": bva,
            "tri": tri,
        })
    return in_maps


def combine_outputs(results, b_out):
    b_out = np.asarray(b_out, dtype=np.float32)
    y = np.empty((B, T, D), dtype=np.float32)
    for b in range(B):
        yt4 = results[2 * b]["ytp"] + results[2 * b + 1]["ytp"]
        yt = yt4.transpose(0, 2, 1, 3).reshape(D, T)  # [cot,m,p,c] -> [D,T]
        y[b] = yt.T + b_out
    return y


def kernel(x, W_qkv, b_qkv, W_out, b_out):
    from concourse.bass_utils import run_bass_kernel_spmd
    if "nc" not in _CACHE:
        _CACHE["nc"] = build_program()
    nc = _CACHE["nc"]
    in_maps = make_in_maps(x, W_qkv, b_qkv, W_out)
    res = run_bass_kernel_spmd(nc, in_maps, list(range(8)))
    return combine_outputs(res.results, b_out)
